# revision 1
# baseline (speedup 1.0000x reference)
"""Trainium2 Bass kernel for nn_AKWA_10118942949711.

4-head channel attention with dilated 3x3 key convs (dilations 1,2,4,8).
Input query (8, 256, 128, 128) f32. Data-parallel: 1 batch per NeuronCore
across 8 cores, no collectives.

Per-core dataflow (all matmuls in float32r = fp32 rounded to 12 mantissa
bits, ~1e-4 rel precision, full TensorE rate; inputs pre-rounded host-side
so plain DMAs produce valid f32r operands):

  - query batch DMA'd into SBUF in a row/col-padded layout: 2 c-blocks of
    (128 part, 144 rows x 130 cols). 8 zero rows top/bottom, 1 zero col
    left/right. 16-row stripes (finer at the start) so conv can begin
    before the full load lands.
  - K = dilated conv, per 3-row group in natural layout (och part, token
    free): 18 PSUM-accumulated matmuls per head into a (64, 3x144) tile;
    the horizontal tap shift goes to the PSUM write offset (even offsets
    only - d=1's odd +-1 shifts are taken on the input side via the 1-col
    pad). Taps reading only zero-padding rows are skipped.
  - K rows are PE-transposed to token-major; Q is computed token-major
    directly (lhsT = x row, rhs = WqT); scores (64x64 per head) accumulate
    in one PSUM bank over all 128 row-chunks.
  - softmax: DVE row-max (negated) -> ACT Exp with accum_out denominator
    -> DVE reciprocal -> normalize probs. Dummy matmuls keep the PE clock
    warm (HAM) through this serial chain.
  - attn is folded into Wv:  awv_h = attn_h @ Wv_h,  ab_h = attn_h @ bv_h,
    so the entire output pass is  out_h = awv_h @ x + ab_h  - 2 matmuls
    per (4-row group, head) reading the padded x directly, with the ab
    bias folded into the PSUM->SBUF copy (split DVE/ACT), then DMA out.

The 1/sqrt(256) score scale is folded into Wq/bq host-side.
"""

import numpy as np

HEADS = 4
DILATIONS = (1, 2, 4, 8)
C = 256
H = 128
W = 128
N = H * W
C4 = C // HEADS

PAD_R = 8            # zero rows above/below
RW = 130             # padded row width (1 + 128 + 1)
NROWS = 144          # padded row count (8 + 128 + 8)
XFREE = NROWS * RW   # free elems per partition per c-block

_CACHE = {}


def _round_f32r(x):
    """Round fp32 to the PE's float32r precision (12-bit mantissa,
    round-to-nearest-even) so plain DMAs can feed f32r matmul operands."""
    u = np.ascontiguousarray(x, np.float32).view(np.uint32)
    r = (u + np.uint32(0x7FF) + ((u >> np.uint32(12)) & np.uint32(1))) \
        & np.uint32(0xFFFFF000)
    return r.view(np.float32)


def _build():
    import concourse.bass as bass
    import concourse.tile as tile
    from concourse import bacc, mybir
    from concourse.masks import make_identity

    f32 = mybir.dt.float32
    f32r = mybir.dt.float32r
    AX = mybir.AxisListType
    OP = mybir.AluOpType
    ACT = mybir.ActivationFunctionType

    nc = bacc.Bacc(target_bir_lowering=False)

    x_ext = nc.declare_dram_parameter("x", [C, H, W], f32r, isOutput=False)
    wqt_ext = nc.declare_dram_parameter("wqt", [2, 128, C], f32r, isOutput=False)
    wkt_ext = nc.declare_dram_parameter("wkt", [HEADS, 9, 2, 128, C4], f32r, isOutput=False)
    wvn_ext = nc.declare_dram_parameter("wvn", [HEADS, C4, C], f32r, isOutput=False)
    bqb_ext = nc.declare_dram_parameter("bqb", [C], f32, isOutput=False)
    bk_ext = nc.declare_dram_parameter("bk", [HEADS, C4], f32, isOutput=False)
    bv_ext = nc.declare_dram_parameter("bv", [HEADS, C4], f32r, isOutput=False)
    out_ext = nc.declare_dram_parameter("out", [C, H, W], f32, isOutput=True)

    with tile.TileContext(nc) as tc:
        with tc.tile_pool(name="persist", bufs=1) as pp:
            # ---- padded x, f32r, 2 c-blocks
            xblk = [pp.tile([128, XFREE], f32r, name=f"xblk{cb}") for cb in range(2)]
            for cb in range(2):
                xb3 = xblk[cb].rearrange("p (r c) -> p r c", c=RW)
                # zero the padding: head rows(+left pad of row 8), row gaps, tail
                nc.vector.memset(xblk[cb][:, 0:PAD_R * RW + 1].bitcast(f32), 0.0)
                gap = bass.AP(
                    tensor=xblk[cb].tensor,
                    offset=xblk[cb].offset + PAD_R * RW + 129,
                    ap=[list(xblk[cb].ap[0]), [RW, H - 1], [1, 2]],
                )
                nc.vector.memset(gap.bitcast(f32), 0.0)
                tail = (H + PAD_R - 1) * RW + 129
                nc.vector.memset(xblk[cb][:, tail:XFREE].bitcast(f32), 0.0)

            # ---- weights (cast to f32r on DMA) and biases
            wk_sb = pp.tile([128, HEADS, 9, 2, C4], f32r, name="wk_sb")
            for h in range(HEADS):
                for cb in range(2):
                    nc.sync.dma_start(
                        out=wk_sb[:, h, :, cb, :],
                        in_=wkt_ext[h, :, cb, :, :].rearrange("t p o -> p t o"),
                    )
            wqt_sb = [pp.tile([128, C], f32r, name=f"wqt{cb}") for cb in range(2)]
            for cb in range(2):
                nc.sync.dma_start(out=wqt_sb[cb], in_=wqt_ext[cb])
            wvn_sb = pp.tile([C4, HEADS, C], f32r, name="wvn_sb")
            for h in range(HEADS):
                nc.sync.dma_start(out=wvn_sb[:, h, :], in_=wvn_ext[h])
            # bq broadcast tile (every partition holds bq[0:256])
            bqb_sb = pp.tile([128, C], f32, name="bqb_sb")
            nc.sync.dma_start(
                out=bqb_sb,
                in_=bass.AP(tensor=bqb_ext, offset=0, ap=[[0, 128], [1, C]]),
            )
            bk_sb = pp.tile([C4, HEADS], f32, name="bk_sb")
            for h in range(HEADS):
                nc.sync.dma_start(
                    out=bk_sb[:, h:h + 1],
                    in_=bass.AP(tensor=bk_ext, offset=h * C4, ap=[[1, C4], [0, 1]]),
                )
            bv_r = pp.tile([C4, HEADS + 1], f32r, name="bv_r")
            nc.vector.memset(bv_r[:, HEADS:HEADS + 1].bitcast(f32), 0.0)
            for h in range(HEADS):
                nc.sync.dma_start(
                    out=bv_r[:, h:h + 1],
                    in_=bass.AP(tensor=bv_ext, offset=h * C4, ap=[[1, C4], [0, 1]]),
                )
            # x stripes: 16 rows per DMA, c-blocks interleaved so early
            # conv groups can start as soon as their rows land
            stripes = [(0, 4), (4, 4), (8, 4), (12, 4)] + \
                      [(16 + 16 * i, 16) for i in range(7)]
            for (sr, sn) in stripes:
                for cb in range(2):
                    xb3s = xblk[cb].rearrange("p (r c) -> p r c", c=RW)
                    nc.sync.dma_start(
                        out=xb3s[:, PAD_R + sr:PAD_R + sr + sn, 1:1 + W],
                        in_=x_ext[cb * 128:(cb + 1) * 128, sr:sr + sn, :],
                    )

            ident_f = pp.tile([64, 64], f32, name="ident_f")
            make_identity(nc, ident_f)
            ident = pp.tile([64, 64], f32r, name="ident")
            nc.vector.tensor_copy(out=ident, in_=ident_f)

            # persistent psum: scores, 4 heads packed on free dim
            with tc.tile_pool(name="scores_ps", bufs=1, space="PSUM") as sp:
                ps_scores = sp.tile([C4, HEADS * C4], f32, name="ps_scores")

                # ================= PASS 1: scores =================
                with tc.tile_pool(name="p1_ps", bufs=1, space="PSUM") as p1ps, \
                     tc.tile_pool(name="p1_sb", bufs=3) as p1sb:
                    first_score = [True]
                    CG = 3  # conv rows per group (psum: 3*144 = 432 <= 512)
                    gr0 = 0
                    while gr0 < H:
                        rsz = min(CG, H - gr0)
                        r0 = PAD_R + gr0
                        # ---- conv k for 4 heads -> psum (64, rsz x 144)
                        ps_k = [p1ps.tile([C4, CG * 144], f32, name=f"ps_k{h}",
                                          tag=f"ps_k{h}") for h in range(HEADS)]
                        for h in range(HEADS):
                            d = DILATIONS[h]
                            k3 = ps_k[h].rearrange("p (r c) -> p r c", c=144)
                            first = True
                            for cb in range(2):
                                for dy in range(3):
                                    rr = r0 + d * (dy - 1)
                                    # rows entirely inside the zero padding
                                    # contribute nothing
                                    if rr + rsz <= PAD_R or rr >= PAD_R + H:
                                        continue
                                    for dx in range(3):
                                        if d == 1:
                                            co = 1 + (dx - 1)   # input-side shift
                                            po = 8
                                        else:
                                            co = 1
                                            po = 8 - d * (dx - 1)
                                        rhs = bass.AP(
                                            tensor=xblk[cb].tensor,
                                            offset=xblk[cb].offset + rr * RW + co,
                                            ap=[list(xblk[cb].ap[0]), [RW, rsz], [1, W]],
                                        ).bitcast(f32r)
                                        nc.tensor.matmul(
                                            k3[:, 0:rsz, po:po + W],
                                            wk_sb[:, h, dy * 3 + dx, cb, :],
                                            rhs,
                                            start=first, stop=False,
                                            skip_group_check=True,
                                        )
                                        first = False
                        # ---- k copy (+bias) to sbuf, per head, f32r
                        k_sb = [p1sb.tile([C4, CG * W], f32r, name=f"k_sb{h}",
                                          tag=f"k_sb{h}") for h in range(HEADS)]
                        for h in range(HEADS):
                            k3 = ps_k[h].rearrange("p (r c) -> p r c", c=144)
                            nc.vector.tensor_scalar_add(
                                out=k_sb[h].rearrange("p (r c) -> p r c", c=W)[:, 0:rsz, :],
                                in0=k3[:, 0:rsz, 8:8 + W],
                                scalar1=bk_sb[:, h:h + 1],
                            )
                        # ---- per-row: qT, k transposes, scores
                        for row in range(rsz):
                            ps_qT = p1ps.tile([128, C], f32, name="ps_qT", tag="ps_qT", bufs=2)
                            for cb in range(2):
                                lhsT = bass.AP(
                                    tensor=xblk[cb].tensor,
                                    offset=xblk[cb].offset + (r0 + row) * RW + 1,
                                    ap=[list(xblk[cb].ap[0]), [1, W]],
                                ).bitcast(f32r)
                                nc.tensor.matmul(ps_qT, lhsT, wqt_sb[cb],
                                                 start=(cb == 0), stop=(cb == 1))
                            q_sb = p1sb.tile([128, C], f32r, name="q_sb", tag="q_sb")
                            nc.vector.tensor_add(out=q_sb, in0=ps_qT, in1=bqb_sb)
                            ps_kT = p1ps.tile([128, HEADS * C4], f32r,
                                              name="ps_kT", tag="ps_kT")
                            for h in range(HEADS):
                                nc.tensor.transpose(
                                    ps_kT[:, h * C4:(h + 1) * C4],
                                    k_sb[h][:, row * W:(row + 1) * W],
                                    ident,
                                )
                            kt = p1sb.tile([128, HEADS * C4], f32r,
                                           name="kt_sb", tag="kt_sb")
                            nc.vector.tensor_copy(out=kt, in_=ps_kT)
                            for h in range(HEADS):
                                nc.tensor.matmul(
                                    ps_scores[:, h * C4:(h + 1) * C4],
                                    q_sb[:, h * C4:(h + 1) * C4],
                                    kt[:, h * C4:(h + 1) * C4],
                                    start=first_score[0], stop=False,
                                    skip_group_check=True,
                                )
                                first_score[0] = False
                        gr0 += rsz

                # ---- PE warm-keeper: dummy matmuls that fill the PE during
                # the softmax/attn@Wv serial chain so HAM stays at full clock
                with tc.tile_pool(name="warm_ps", bufs=1, space="PSUM") as wps:
                    scratch = wps.tile([C4, 512], f32, name="scratch")
                    for i_ in range(16):
                        nc.tensor.matmul(scratch, wqt_sb[0][:, 0:C4],
                                         xblk[0][:, 0:512], start=True, stop=True,
                                         skip_group_check=True)

                    # ================= softmax =================
                    negmax = pp.tile([C4, HEADS], f32, name="negmax")
                    probs = pp.tile([C4, HEADS * C4], f32, name="probs")
                    denom = pp.tile([C4, HEADS], f32, name="denom")
                    rden = pp.tile([C4, HEADS], f32, name="rden")
                    probs_n = pp.tile([C4, HEADS * C4], f32r, name="probs_n")
                    attnT_sb = pp.tile([C4, HEADS * C4], f32r, name="attnT_sb")
                    for h in range(HEADS):
                        sl = slice(h * C4, (h + 1) * C4)
                        nc.vector.tensor_reduce(
                            out=negmax[:, h:h + 1], in_=ps_scores[:, sl],
                            axis=AX.X, op=OP.max, negate=True,
                        )
                        nc.scalar.activation(
                            out=probs[:, sl], in_=ps_scores[:, sl], func=ACT.Exp,
                            bias=negmax[:, h:h + 1], scale=1.0,
                            accum_out=denom[:, h:h + 1],
                        )
                    nc.vector.reciprocal(out=rden, in_=denom)
                    for h in range(HEADS):
                        sl = slice(h * C4, (h + 1) * C4)
                        nc.vector.tensor_scalar_mul(
                            out=probs_n[:, sl], in0=probs[:, sl],
                            scalar1=rden[:, h:h + 1],
                        )

                    # ---- fold attn into Wv:  awv_h = attn_h @ Wv_h  (64 x 256)
                    # and ab_h = attn_h @ bv_h (64), then
                    # out_h = awv_h @ x + ab_h
                    awvT_sb = pp.tile([128, 2, HEADS, C4], f32r, name="awvT_sb")
                    ab_sb = pp.tile([C4, 4 * HEADS], f32, name="ab_sb")
                    with tc.tile_pool(name="awv_ps", bufs=2, space="PSUM") as aps:
                        ps_attnT = aps.tile([C4, HEADS * C4], f32r, name="ps_attnT",
                                            tag="awv")
                        for h in range(HEADS):
                            sl = slice(h * C4, (h + 1) * C4)
                            nc.tensor.transpose(ps_attnT[:, sl], probs_n[:, sl], ident)
                        nc.vector.tensor_copy(out=attnT_sb, in_=ps_attnT)
                        ps_ab = aps.tile([C4, 4 * HEADS], f32, name="ps_ab", tag="awv")
                        for h in range(HEADS):
                            nc.tensor.matmul(
                                ps_ab[:, 4 * h:4 * h + 2],
                                attnT_sb[:, h * C4:(h + 1) * C4],
                                bv_r[:, h:h + 2],
                                start=(h == 0), stop=(h == 3),
                                skip_group_check=True,
                            )
                        nc.vector.tensor_copy(out=ab_sb, in_=ps_ab)
                        awv_sb = pp.tile([C4, HEADS * C], f32r, name="awv_sb")
                        for h in range(HEADS):
                            ps_awv = aps.tile([C4, C], f32, name="ps_awv", tag="awv")
                            nc.tensor.matmul(
                                ps_awv,
                                attnT_sb[:, h * C4:(h + 1) * C4],
                                wvn_sb[:, h, :],
                                start=True, stop=True,
                            )
                            nc.vector.tensor_copy(out=awv_sb[:, h * C:(h + 1) * C],
                                                  in_=ps_awv)
                        for h in range(HEADS):
                            ps_awvT = aps.tile([128, HEADS * C4], f32r,
                                               name="ps_awvT", tag="awv")
                            for cb in range(2):
                                nc.tensor.transpose(
                                    ps_awvT[:, cb * C4:(cb + 1) * C4],
                                    awv_sb[:, h * C + cb * 128:h * C + (cb + 1) * 128],
                                    ident,
                                )
                            for cb in range(2):
                                nc.vector.tensor_copy(
                                    out=awvT_sb[:, cb, h, :],
                                    in_=ps_awvT[:, cb * C4:(cb + 1) * C4],
                                )

                # ================= PASS 2: out = awv @ x =================
                with tc.tile_pool(name="p2_ps", bufs=6, space="PSUM") as p2ps, \
                     tc.tile_pool(name="p2_sb", bufs=3) as p2sb:
                    out_flat = out_ext.rearrange("c h w -> c (h w)")
                    NG2 = H // 4
                    for g in range(NG2):
                        r0 = PAD_R + 4 * g
                        for h in range(HEADS):
                            ps_o = p2ps.tile([C4, 4 * W], f32, name=f"ps_o{h}",
                                             tag="p2ps")
                            for cb in range(2):
                                rhs = bass.AP(
                                    tensor=xblk[cb].tensor,
                                    offset=xblk[cb].offset + r0 * RW + 1,
                                    ap=[list(xblk[cb].ap[0]), [RW, 4], [1, W]],
                                ).bitcast(f32r)
                                nc.tensor.matmul(
                                    ps_o,
                                    awvT_sb[:, cb, h, :],
                                    rhs,
                                    start=(cb == 0), stop=(cb == 1),
                                )
                            os = p2sb.tile([C4, 4 * W], f32, name=f"o_sb{h}",
                                           tag=f"o_sb{h}", bufs=3)
                            if h < 2:
                                nc.scalar.add(out=os, in_=ps_o,
                                              add=ab_sb[:, 4 * h:4 * h + 1])
                            else:
                                nc.vector.tensor_scalar_add(
                                    out=os, in0=ps_o,
                                    scalar1=ab_sb[:, 4 * h:4 * h + 1],
                                )
                            nc.sync.dma_start(
                                out=out_flat[h * C4:(h + 1) * C4,
                                             g * 4 * W:(g + 1) * 4 * W],
                                in_=os,
                            )

    nc.finalize()
    return nc


def _prep_weights(Wq, bq, Wk, bk, Wv, bv):
    s = np.float32(1.0 / 16.0)  # 1/sqrt(256) folded into Wq, bq
    wqt = _round_f32r(
        (Wq * s).transpose(2, 0, 1).reshape(C, C).reshape(2, 128, C))
    # wkt[h, tap, cb, c', o] = Wk[h, o, cb*128+c', dy, dx]
    wkt = _round_f32r(
        Wk.transpose(0, 3, 4, 2, 1).reshape(HEADS, 9, 2, 128, C4))
    wvn = _round_f32r(Wv)  # (H, C4, C) natural
    bqb = np.ascontiguousarray((bq * s).reshape(C), np.float32)
    bkf = np.ascontiguousarray(bk, np.float32)
    bvf = _round_f32r(bv.reshape(HEADS, C4))
    return {"wqt": wqt, "wkt": wkt, "wvn": wvn, "bqb": bqb, "bk": bkf, "bv": bvf}


def run(inputs, trace=False, trace_kwargs=None):
    """Run on 8 cores; returns (output (8,C,H,W) f32, BassKernelResults)."""
    from concourse.bass_utils import run_bass_kernel_spmd

    if "nc" not in _CACHE:
        _CACHE["nc"] = _build()
    nc = _CACHE["nc"]

    query = _round_f32r(inputs["query"])
    w = _prep_weights(inputs["Wq"], inputs["bq"], inputs["Wk"],
                      inputs["bk"], inputs["Wv"], inputs["bv"])
    core_ids = list(range(8))
    in_maps = [dict(w, x=np.ascontiguousarray(query[i])) for i in core_ids]
    res = run_bass_kernel_spmd(nc, in_maps, core_ids, trace=trace,
                               **(trace_kwargs or {}))
    out = np.stack([res.results[i]["out"] for i in core_ids])
    return out, res


def kernel(**inputs) -> np.ndarray:
    out, _ = run(inputs, trace=False)
    return out



# revision 9
# speedup vs baseline: 1.2773x; 1.2773x over previous
"""Trainium2 Bass kernel for nn_AKWA_10118942949711.

4-head channel attention with dilated 3x3 key convs (dilations 1,2,4,8).
Input query (8, 256, 128, 128) f32. Data-parallel: 1 batch per NeuronCore
across 8 cores, no collectives.

Per-core dataflow (all matmuls in float32r = fp32 rounded to 12 mantissa
bits, full TensorE rate at moving-size >= 256; inputs pre-rounded
host-side so plain DMAs produce valid f32r operands):

  - query batch DMA'd into SBUF in a row/col-padded layout: 2 c-blocks of
    (128 part, 144 rows x 130 cols), 16-row stripes.
  - K = dilated conv, per 3-row group in natural layout (och part, token
    free): 18 PSUM-accumulated matmuls per head into a (64, 3x144) tile;
    horizontal tap shift goes to the PSUM write offset (even offsets
    only - d=1's odd +-1 shifts are taken on the input side via the
    1-col pad). Taps reading only zero-padding rows are skipped.
  - K rows are PE-transposed to token-major; Q is computed token-major
    directly (lhsT = x row, rhs = WqT).
  - scores: f32r matmuls below 256 moving elems run at 1/4 rate, so
    instead of 4 per-head (64x64, N=64) matmuls per row we issue 2
    (128x256, N=256) matmuls whose useful output is the diagonal
    64x64 blocks (lhsT = q och-pair, rhs = kT all heads); the garbage
    off-diagonal blocks land in unused PSUM.  Head h accumulates at
    sc2[(h%2)*64:, ...] - odd heads live on partitions 64-127.
  - softmax per head on its quadrant: DVE row-max (negated) -> ACT Exp
    with accum_out denominator -> DVE reciprocal -> normalize. Dummy
    matmuls keep the PE clock warm (HAM) through the serial chain.
    attnT transposes bring odd heads back to partitions 0-63 (row-
    tiled transpose sources are legal; PSUM matmul dst must stay at
    partition 0 - this toolchain cannot column-tile).
  - attn folded into Wv:  awv_h = attn_h @ Wv_h,  ab_h = attn_h @ bv_h,
    so the output pass is  out_h = awv_h @ x + ab_h.  awvT is laid out
    pair-adjacent so PASS 2 runs one (128, 512) matmul per (4-row
    group, head pair, c-block): heads share the same x rhs, so the
    pair stacks to M=128 and fills the whole PE array.  ab is moved to
    the paired 128-partition layout with a tiny SBUF->SBUF DMA; the
    pair bias folds into the PSUM->SBUF copy (pair0 on DVE, pair1 on
    ACT), then one contiguous 128-channel DMA out per pair.

The 1/sqrt(256) score scale is folded into Wq/bq host-side.
"""

import numpy as np

HEADS = 4
DILATIONS = (1, 2, 4, 8)
C = 256
H = 128
W = 128
N = H * W
C4 = C // HEADS

PAD_R = 8            # zero rows above/below
RW = 130             # padded row width (1 + 128 + 1)
NROWS = 144          # padded row count (8 + 128 + 8)
XFREE = NROWS * RW   # free elems per partition per c-block

_CACHE = {}


def _round_f32r(x):
    """Round fp32 to the PE's float32r precision (12-bit mantissa,
    round-to-nearest-even) so plain DMAs can feed f32r matmul operands."""
    u = np.ascontiguousarray(x, np.float32).view(np.uint32)
    r = (u + np.uint32(0x7FF) + ((u >> np.uint32(12)) & np.uint32(1))) \
        & np.uint32(0xFFFFF000)
    return r.view(np.float32)


def _build():
    import concourse.bass as bass
    import concourse.tile as tile
    from concourse import bacc, mybir
    from concourse.masks import make_identity

    f32 = mybir.dt.float32
    f32r = mybir.dt.float32r
    AX = mybir.AxisListType
    OP = mybir.AluOpType
    ACT = mybir.ActivationFunctionType

    nc = bacc.Bacc(target_bir_lowering=False)

    x_ext = nc.declare_dram_parameter("x", [C, H, W], f32r, isOutput=False)
    wqt_ext = nc.declare_dram_parameter("wqt", [2, 128, C], f32r, isOutput=False)
    wkt_ext = nc.declare_dram_parameter("wkt", [HEADS, 9, 2, 128, C4], f32r, isOutput=False)
    wvn_ext = nc.declare_dram_parameter("wvn", [HEADS, C4, C], f32r, isOutput=False)
    bqb_ext = nc.declare_dram_parameter("bqb", [C], f32, isOutput=False)
    bk_ext = nc.declare_dram_parameter("bk", [HEADS, C4], f32, isOutput=False)
    bv_ext = nc.declare_dram_parameter("bv", [HEADS, C4], f32r, isOutput=False)
    out_ext = nc.declare_dram_parameter("out", [C, H, W], f32, isOutput=True)

    with tile.TileContext(nc) as tc:
        with tc.tile_pool(name="persist", bufs=1) as pp:
            # ---- padded x, f32r, 2 c-blocks
            xblk = [pp.tile([128, XFREE], f32r, name=f"xblk{cb}") for cb in range(2)]
            for cb in range(2):
                # zero the padding: head rows(+left pad of row 8), row gaps, tail
                nc.vector.memset(xblk[cb][:, 0:PAD_R * RW + 1].bitcast(f32), 0.0)
                gap = bass.AP(
                    tensor=xblk[cb].tensor,
                    offset=xblk[cb].offset + PAD_R * RW + 129,
                    ap=[list(xblk[cb].ap[0]), [RW, H - 1], [1, 2]],
                )
                nc.vector.memset(gap.bitcast(f32), 0.0)
                tail = (H + PAD_R - 1) * RW + 129
                nc.vector.memset(xblk[cb][:, tail:XFREE].bitcast(f32), 0.0)

            # ---- weights (f32r host-prepped) and biases
            wk_sb = pp.tile([128, HEADS, 9, 2, C4], f32r, name="wk_sb")
            for h in range(HEADS):
                for cb in range(2):
                    nc.sync.dma_start(
                        out=wk_sb[:, h, :, cb, :],
                        in_=wkt_ext[h, :, cb, :, :].rearrange("t p o -> p t o"),
                    )
            wqt_sb = [pp.tile([128, C], f32r, name=f"wqt{cb}") for cb in range(2)]
            for cb in range(2):
                nc.sync.dma_start(out=wqt_sb[cb], in_=wqt_ext[cb])
            wvn_sb = pp.tile([C4, HEADS, C], f32r, name="wvn_sb")
            for h in range(HEADS):
                nc.sync.dma_start(out=wvn_sb[:, h, :], in_=wvn_ext[h])
            # bq broadcast tile (every partition holds bq[0:256])
            bqb_sb = pp.tile([128, C], f32, name="bqb_sb")
            nc.sync.dma_start(
                out=bqb_sb,
                in_=bass.AP(tensor=bqb_ext, offset=0, ap=[[0, 128], [1, C]]),
            )
            bk_sb = pp.tile([C4, HEADS], f32, name="bk_sb")
            for h in range(HEADS):
                nc.sync.dma_start(
                    out=bk_sb[:, h:h + 1],
                    in_=bass.AP(tensor=bk_ext, offset=h * C4, ap=[[1, C4], [0, 1]]),
                )
            bv_r = pp.tile([C4, HEADS + 1], f32r, name="bv_r")
            nc.vector.memset(bv_r[:, HEADS:HEADS + 1].bitcast(f32), 0.0)
            for h in range(HEADS):
                nc.sync.dma_start(
                    out=bv_r[:, h:h + 1],
                    in_=bass.AP(tensor=bv_ext, offset=h * C4, ap=[[1, C4], [0, 1]]),
                )
            # x stripes: 16 rows per DMA, c-blocks interleaved so early
            # conv groups can start as soon as their rows land
            stripes = [(0, 4), (4, 4), (8, 4), (12, 4)] + \
                      [(16 + 16 * i, 16) for i in range(7)]
            for (sr, sn) in stripes:
                for cb in range(2):
                    xb3s = xblk[cb].rearrange("p (r c) -> p r c", c=RW)
                    nc.sync.dma_start(
                        out=xb3s[:, PAD_R + sr:PAD_R + sr + sn, 1:1 + W],
                        in_=x_ext[cb * 128:(cb + 1) * 128, sr:sr + sn, :],
                    )

            # identity in both partition halves (odd heads' attnT transposes
            # read probs at partitions 64-127)
            ident_f = pp.tile([64, 64], f32, name="ident_f")
            make_identity(nc, ident_f)
            ident2 = pp.tile([128, 64], f32r, name="ident2")
            nc.vector.tensor_copy(out=ident2[0:64, :], in_=ident_f)
            nc.sync.dma_start(out=ident2[64:128, :], in_=ident2[0:64, :])

            # persistent psum: scores. Layout (128, 512): pair p matmul
            # writes [:, p*256:(p+1)*256]; useful head blocks are
            # h=2p:   [0:64,    p*256 + 2p*64 : p*256 + (2p+1)*64]
            # h=2p+1: [64:128,  p*256 + (2p+1)*64 : p*256 + (2p+2)*64]
            with tc.tile_pool(name="scores_ps", bufs=1, space="PSUM") as sp:
                sc2 = sp.tile([128, 512], f32, name="sc2")

                def sc_quad(h):
                    p, half = h // 2, h % 2
                    return sc2[64 * half:64 * (half + 1),
                               p * 256 + h * C4:p * 256 + (h + 1) * C4]

                # ================= PASS 1: scores =================
                with tc.tile_pool(name="p1_ps", bufs=1, space="PSUM") as p1ps, \
                     tc.tile_pool(name="p1_sb", bufs=3) as p1sb:
                    first_sc = [True]
                    CG = 3  # conv rows per group (psum: 3*144 = 432 <= 512)
                    gr0 = 0
                    while gr0 < H:
                        rsz = min(CG, H - gr0)
                        r0 = PAD_R + gr0
                        # ---- conv k for 4 heads -> psum (64, rsz x 144)
                        ps_k = [p1ps.tile([C4, CG * 144], f32, name=f"ps_k{h}",
                                          tag=f"ps_k{h}") for h in range(HEADS)]
                        for h in range(HEADS):
                            d = DILATIONS[h]
                            k3 = ps_k[h].rearrange("p (r c) -> p r c", c=144)
                            first = True
                            for cb in range(2):
                                for dy in range(3):
                                    rr = r0 + d * (dy - 1)
                                    # rows entirely inside the zero padding
                                    # contribute nothing
                                    if rr + rsz <= PAD_R or rr >= PAD_R + H:
                                        continue
                                    for dx in range(3):
                                        if d == 1:
                                            co = 1 + (dx - 1)   # input-side shift
                                            po = 8
                                        else:
                                            co = 1
                                            po = 8 - d * (dx - 1)
                                        rhs = bass.AP(
                                            tensor=xblk[cb].tensor,
                                            offset=xblk[cb].offset + rr * RW + co,
                                            ap=[list(xblk[cb].ap[0]), [RW, rsz], [1, W]],
                                        ).bitcast(f32r)
                                        nc.tensor.matmul(
                                            k3[:, 0:rsz, po:po + W],
                                            wk_sb[:, h, dy * 3 + dx, cb, :],
                                            rhs,
                                            start=first, stop=False,
                                            skip_group_check=True,
                                        )
                                        first = False
                        # ---- k copy (+bias) to sbuf, per head, f32r
                        k_sb = [p1sb.tile([C4, CG * W], f32r, name=f"k_sb{h}",
                                          tag=f"k_sb{h}", bufs=2) for h in range(HEADS)]
                        for h in range(HEADS):
                            k3 = ps_k[h].rearrange("p (r c) -> p r c", c=144)
                            nc.vector.tensor_scalar_add(
                                out=k_sb[h].rearrange("p (r c) -> p r c", c=W)[:, 0:rsz, :],
                                in0=k3[:, 0:rsz, 8:8 + W],
                                scalar1=bk_sb[:, h:h + 1],
                            )
                        # ---- per-row: qT, k transposes, scores
                        for row in range(rsz):
                            # union bank: qT in [0:256] (f32), kT in [256:512]
                            ps_qk = p1ps.tile([128, 512], f32, name="ps_qk",
                                              tag="ps_qk", bufs=2)
                            for cb in range(2):
                                lhsT = bass.AP(
                                    tensor=xblk[cb].tensor,
                                    offset=xblk[cb].offset + (r0 + row) * RW + 1,
                                    ap=[list(xblk[cb].ap[0]), [1, W]],
                                ).bitcast(f32r)
                                nc.tensor.matmul(ps_qk[:, 0:C], lhsT, wqt_sb[cb],
                                                 start=(cb == 0), stop=(cb == 1),
                                                 skip_group_check=True)
                            q_sb = p1sb.tile([128, C], f32r, name="q_sb", tag="q_sb")
                            nc.vector.tensor_add(out=q_sb, in0=ps_qk[:, 0:C],
                                                 in1=bqb_sb)
                            kT = ps_qk[:, C:2 * C].bitcast(f32r)
                            for h in range(HEADS):
                                nc.tensor.transpose(
                                    kT[:, h * C4:(h + 1) * C4],
                                    k_sb[h][:, row * W:(row + 1) * W],
                                    ident2[0:64, :],
                                )
                            kt = p1sb.tile([128, C], f32r, name="kt_sb", tag="kt_sb")
                            nc.scalar.activation(out=kt, in_=kT, func=ACT.Copy)
                            # scores: one N=256 matmul per head pair (f32r needs
                            # moving >= 256 for full rate); off-diagonal blocks
                            # are garbage
                            for p in range(2):
                                nc.tensor.matmul(
                                    sc2[:, p * 256:(p + 1) * 256],
                                    q_sb[:, p * 128:(p + 1) * 128],
                                    kt,
                                    start=first_sc[0], stop=False,
                                    skip_group_check=True,
                                )
                                first_sc[0] = False
                        gr0 += rsz

                # ---- PE warm-keeper: dummy matmuls that fill the PE during
                # the softmax/attn@Wv serial chain so HAM stays at full clock
                with tc.tile_pool(name="warm_ps", bufs=1, space="PSUM") as wps:
                    scratch = wps.tile([C4, 512], f32, name="scratch")
                    for i_ in range(16):
                        nc.tensor.matmul(scratch, wqt_sb[0][:, 0:C4],
                                         xblk[0][:, 0:512], start=True, stop=True,
                                         skip_group_check=True)

                    # ================= softmax (per-head quadrants) ==========
                    # head h lives on partitions (h%2)*64:(h%2+1)*64
                    negmax = pp.tile([128, 2], f32, name="negmax")
                    probs = pp.tile([128, 2 * C4], f32, name="probs")
                    denom = pp.tile([128, 2], f32, name="denom")
                    rden = pp.tile([128, 2], f32, name="rden")
                    probs_n = pp.tile([128, 2 * C4], f32r, name="probs_n")
                    attnT_sb = pp.tile([C4, HEADS * C4], f32r, name="attnT_sb")
                    for h in range(HEADS):
                        p, half = h // 2, h % 2
                        hsl = slice(64 * half, 64 * (half + 1))
                        nc.vector.tensor_reduce(
                            out=negmax[hsl, p:p + 1], in_=sc_quad(h),
                            axis=AX.X, op=OP.max, negate=True,
                        )
                        nc.scalar.activation(
                            out=probs[hsl, p * C4:(p + 1) * C4], in_=sc_quad(h),
                            func=ACT.Exp,
                            bias=negmax[hsl, p:p + 1], scale=1.0,
                            accum_out=denom[hsl, p:p + 1],
                        )
                    nc.vector.reciprocal(out=rden, in_=denom)
                    for h in range(HEADS):
                        p, half = h // 2, h % 2
                        hsl = slice(64 * half, 64 * (half + 1))
                        nc.vector.tensor_scalar_mul(
                            out=probs_n[hsl, p * C4:(p + 1) * C4],
                            in0=probs[hsl, p * C4:(p + 1) * C4],
                            scalar1=rden[hsl, p:p + 1],
                        )

                    # ---- fold attn into Wv:  awv_h = attn_h @ Wv_h  (64 x 256)
                    # and ab_h = attn_h @ bv_h (64), then
                    # out_h = awv_h @ x + ab_h
                    # awvT pair-adjacent: [c', cb, h, och] so a (128, 128)
                    # slice [:, cb, 2p:2p+2, :] is the pass-2 lhsT
                    awvT_sb = pp.tile([128, 2, HEADS, C4], f32r, name="awvT_sb")
                    ab_sb = pp.tile([C4, 4 * HEADS], f32, name="ab_sb")
                    ab2 = pp.tile([128, 2], f32, name="ab2")
                    with tc.tile_pool(name="awv_ps", bufs=2, space="PSUM") as aps:
                        ps_attnT = aps.tile([C4, HEADS * C4], f32r, name="ps_attnT",
                                            tag="awv")
                        for h in range(HEADS):
                            p, half = h // 2, h % 2
                            hsl = slice(64 * half, 64 * (half + 1))
                            nc.tensor.transpose(
                                ps_attnT[:, h * C4:(h + 1) * C4],
                                probs_n[hsl, p * C4:(p + 1) * C4],
                                ident2[hsl, :],
                            )
                        nc.vector.tensor_copy(out=attnT_sb, in_=ps_attnT)
                        ps_ab = aps.tile([C4, 4 * HEADS], f32, name="ps_ab", tag="awv")
                        for h in range(HEADS):
                            nc.tensor.matmul(
                                ps_ab[:, 4 * h:4 * h + 2],
                                attnT_sb[:, h * C4:(h + 1) * C4],
                                bv_r[:, h:h + 2],
                                start=(h == 0), stop=(h == 3),
                                skip_group_check=True,
                            )
                        nc.vector.tensor_copy(out=ab_sb, in_=ps_ab)
                        # ab to the paired 128-partition layout (odd heads to
                        # partitions 64-127) via tiny SBUF->SBUF DMAs
                        for h in range(HEADS):
                            p, half = h // 2, h % 2
                            nc.sync.dma_start(
                                out=ab2[64 * half:64 * (half + 1), p:p + 1],
                                in_=ab_sb[:, 4 * h:4 * h + 1],
                            )
                        awv_sb = pp.tile([C4, HEADS * C], f32r, name="awv_sb")
                        for h in range(HEADS):
                            ps_awv = aps.tile([C4, C], f32, name="ps_awv", tag="awv")
                            nc.tensor.matmul(
                                ps_awv,
                                attnT_sb[:, h * C4:(h + 1) * C4],
                                wvn_sb[:, h, :],
                                start=True, stop=True,
                            )
                            nc.vector.tensor_copy(out=awv_sb[:, h * C:(h + 1) * C],
                                                  in_=ps_awv)
                        for h in range(HEADS):
                            ps_awvT = aps.tile([128, HEADS * C4], f32r,
                                               name="ps_awvT", tag="awv")
                            for cb in range(2):
                                nc.tensor.transpose(
                                    ps_awvT[:, cb * C4:(cb + 1) * C4],
                                    awv_sb[:, h * C + cb * 128:h * C + (cb + 1) * 128],
                                    ident2[0:64, :],
                                )
                            for cb in range(2):
                                nc.vector.tensor_copy(
                                    out=awvT_sb[:, cb, h, :],
                                    in_=ps_awvT[:, cb * C4:(cb + 1) * C4],
                                )

                # ================= PASS 2: out = awv @ x =================
                # head pairs share the x rhs -> stack them to M=128 (full
                # array): one matmul per (group, pair, cb)
                with tc.tile_pool(name="p2_ps", bufs=6, space="PSUM") as p2ps, \
                     tc.tile_pool(name="p2_sb", bufs=3) as p2sb:
                    out_flat = out_ext.rearrange("c h w -> c (h w)")
                    NG2 = H // 4
                    for g in range(NG2):
                        r0 = PAD_R + 4 * g
                        for p in range(2):
                            ps_o = p2ps.tile([128, 4 * W], f32, name=f"ps_o{p}",
                                             tag="p2ps")
                            for cb in range(2):
                                rhs = bass.AP(
                                    tensor=xblk[cb].tensor,
                                    offset=xblk[cb].offset + r0 * RW + 1,
                                    ap=[list(xblk[cb].ap[0]), [RW, 4], [1, W]],
                                ).bitcast(f32r)
                                nc.tensor.matmul(
                                    ps_o,
                                    awvT_sb[:, cb, 2 * p:2 * p + 2, :]
                                        .rearrange("c h o -> c (h o)"),
                                    rhs,
                                    start=(cb == 0), stop=(cb == 1),
                                    skip_group_check=True,
                                )
                            os = p2sb.tile([128, 4 * W], f32, name=f"o_sb{p}",
                                           tag=f"o_sb{p}", bufs=3)
                            if p == 0:
                                nc.vector.tensor_scalar_add(
                                    out=os, in0=ps_o, scalar1=ab2[:, 0:1],
                                )
                            else:
                                nc.scalar.add(out=os, in_=ps_o, add=ab2[:, 1:2])
                            nc.sync.dma_start(
                                out=out_flat[p * 128:(p + 1) * 128,
                                             g * 4 * W:(g + 1) * 4 * W],
                                in_=os,
                            )

    nc.finalize()
    return nc


def _prep_weights(Wq, bq, Wk, bk, Wv, bv):
    s = np.float32(1.0 / 16.0)  # 1/sqrt(256) folded into Wq, bq
    wqt = _round_f32r(
        (Wq * s).transpose(2, 0, 1).reshape(C, C).reshape(2, 128, C))
    # wkt[h, tap, cb, c', o] = Wk[h, o, cb*128+c', dy, dx]
    wkt = _round_f32r(
        Wk.transpose(0, 3, 4, 2, 1).reshape(HEADS, 9, 2, 128, C4))
    wvn = _round_f32r(Wv)  # (H, C4, C) natural
    bqb = np.ascontiguousarray((bq * s).reshape(C), np.float32)
    bkf = np.ascontiguousarray(bk, np.float32)
    bvf = _round_f32r(bv.reshape(HEADS, C4))
    return {"wqt": wqt, "wkt": wkt, "wvn": wvn, "bqb": bqb, "bk": bkf, "bv": bvf}


def run(inputs, trace=False, trace_kwargs=None):
    """Run on 8 cores; returns (output (8,C,H,W) f32, BassKernelResults)."""
    from concourse.bass_utils import run_bass_kernel_spmd

    if "nc" not in _CACHE:
        _CACHE["nc"] = _build()
    nc = _CACHE["nc"]

    query = _round_f32r(inputs["query"])
    w = _prep_weights(inputs["Wq"], inputs["bq"], inputs["Wk"],
                      inputs["bk"], inputs["Wv"], inputs["bv"])
    core_ids = list(range(8))
    in_maps = [dict(w, x=np.ascontiguousarray(query[i])) for i in core_ids]
    res = run_bass_kernel_spmd(nc, in_maps, core_ids, trace=trace,
                               **(trace_kwargs or {}))
    out = np.stack([res.results[i]["out"] for i in core_ids])
    return out, res


def kernel(**inputs) -> np.ndarray:
    out, _ = run(inputs, trace=False)
    return out
